# revision 12
# baseline (speedup 1.0000x reference)
"""GCN (2-layer) Trainium2 kernel over 8 NeuronCores — v2.

Strategy (dst-sharded pull-gather, bf16):
- Nodes sharded contiguously: core i owns nodes [6250*i, 6250*(i+1)).
- Layer table rows are bf16, PADDED to 128 values (64 real + 64 unread) so
  every row is one 256-byte gather element (dma_gather requires 256B-aligned
  elements; gather cost is per-descriptor latency, so padding is free).
- The table is AllGather'd in two halves (src tiles 0:25 / 25:49) so the
  second half's collective overlaps the first half's gather+aggregate pass.
- Aggregation per 128-dst tile: gathered 128-edge blocks are summed with a
  TensorE matmul against a DVE-built one-hot selector S (bf16).
- Self-loop terms are folded into the per-tile tail math (never gathered).
- Gathers run in 2048-index chunks round-robined over 4 SWDGE queues.
- Layer 2 aggregates the (dinv*relu(.)) table, then applies W2 after a PE
  transpose; log_softmax on ACT+DVE.
The edge structure is baked into the program; block counts are padded to the
max over cores so the SPMD program is identical on all 8 cores.
"""

import numpy as np

try:
    from ml_dtypes import bfloat16 as bf16np
except ImportError:  # pragma: no cover
    bf16np = None

N_NODES = 50000
CORES = 8
SH = 6250          # owned nodes per core
SHP = 6272         # padded shard rows (49*128)
NT = 49            # dst tiles per core
BLK = 128
F0, F1, F2 = 96, 64, 16
TA = 25            # tiles in half A
TB = NT - TA       # 24
WA, WB = TA * F1, TB * F1            # compact cols per half: 1600 / 1536
EA, EB = 1024 * WA // BLK, 1024 * WB // BLK  # 256B elements per half: 12800 / 12288
CHUNK = 1024
CB = CHUNK // BLK  # blocks per chunk
NQ = 4             # SWDGE queues
PAD_DL = 300.0     # is_equal miss => zero S row
LOOKAHEAD = 20


def _bf(x):
    return np.asarray(x, np.float32).astype(bf16np)


def host_prep(x, edge_index, W1, b1, W2, b2):
    src = np.asarray(edge_index[0], dtype=np.int64)
    dst = np.asarray(edge_index[1], dtype=np.int64)
    deg_full = np.bincount(dst, minlength=N_NODES).astype(np.float32) + 1.0

    # split edges by dst shard
    order = np.argsort(dst, kind="stable")
    s_sorted, d_sorted = src[order], dst[order]
    bounds = np.searchsorted(d_sorted, np.arange(0, N_NODES + 1, SH))

    # per-core, per-pass, per-tile edge lists: (elem, dl)
    counts = np.zeros((CORES, 2, NT), np.int64)
    lists = [[[None] * NT for _ in range(2)] for _ in range(CORES)]
    for i in range(CORES):
        es = s_sorted[bounds[i]:bounds[i + 1]]
        ed = d_sorted[bounds[i]:bounds[i + 1]] - SH * i
        s_sh = es // SH
        l = es - SH * s_sh
        ts = l // BLK
        p = l - ts * BLK
        half = (ts >= TA).astype(np.int64)
        rowA = WA * (s_sh * BLK + p) + F1 * ts          # bf16 units
        rowB = WB * (s_sh * BLK + p) + F1 * (ts - TA)
        elem = np.where(half == 0, rowA // BLK, rowB // BLK)
        par = np.where(half == 0, (rowA // F1) % 2, (rowB // F1) % 2)
        t = ed // BLK
        dl = ed - t * BLK + BLK * par                    # 0..255
        key = half * NT + t
        o = np.argsort(key, kind="stable")
        key_s, elem_s, dl_s = key[o], elem[o], dl[o]
        kb = np.searchsorted(key_s, np.arange(2 * NT + 1))
        for P in range(2):
            for tt in range(NT):
                a, b = kb[P * NT + tt], kb[P * NT + tt + 1]
                lists[i][P][tt] = (elem_s[a:b], dl_s[a:b])
                counts[i, P, tt] = b - a

    # uniform block counts across cores
    B = np.maximum(1, -(-counts.max(axis=0) // BLK))  # [2, NT]
    nblk = [int(B[P].sum()) for P in range(2)]
    nch = [-(-nblk[P] // CB) for P in range(2)]

    data = []
    for i in range(CORES):
        d = dict()
        for P in range(2):
            els, dls = [], []
            for tt in range(NT):
                e, q = lists[i][P][tt]
                pad = int(B[P, tt]) * BLK - len(e)
                els.append(np.concatenate([e, np.zeros(pad, np.int64)]))
                dls.append(np.concatenate([q.astype(np.float32),
                                           np.full(pad, PAD_DL, np.float32)]))
            estream = np.concatenate(els)
            dstream = np.concatenate(dls)
            tail = nch[P] * CHUNK - len(estream)
            estream = np.concatenate([estream, np.zeros(tail, np.int64)])
            # idx plane [128, nch*128]: idx j at [j%16, j//16], replicated x8
            pl = estream.reshape(-1, 16).T.astype(np.int16)
            d[f"idx{P}"] = np.ascontiguousarray(np.tile(pl, (8, 1)))
            # dl plane [128, nblk]
            d[f"dl{P}"] = np.ascontiguousarray(
                dstream.reshape(-1, BLK).T.astype(np.float32))
        degp = np.ones((BLK, NT), np.float32)
        dsh = deg_full[SH * i:SH * (i + 1)]
        dp = np.concatenate([dsh, np.ones(SHP - SH, np.float32)])
        degp[:, :] = dp.reshape(NT, BLK).T
        d["deg"] = np.ascontiguousarray(degp)
        xs = np.zeros((F0, SHP), np.float32)
        xs[:, :SH] = np.asarray(x[SH * i:SH * (i + 1)], np.float32).T
        d["xT"] = np.ascontiguousarray(_bf(xs))
        data.append(d)

    ident = np.eye(BLK, dtype=np.float32)
    consts = dict(
        W1=_bf(W1), W2=_bf(W2),
        b1b=np.tile(np.asarray(b1, np.float32), (BLK, 1)),
        b2b=np.tile(np.asarray(b2, np.float32), (BLK, 1)),
        iota=_bf(np.tile(np.arange(2 * BLK, dtype=np.float32), (BLK, 1))),
        ident=ident,
    )
    meta = dict(B=B, nblk=nblk, nch=nch)
    return data, consts, meta


def numpy_sim(x, edge_index, W1, b1, W2, b2):
    """Emulate the device numerics (bf16 tables/weights) edge-wise."""
    def f(a):
        return _bf(a).astype(np.float32)

    src = np.asarray(edge_index[0]); dst = np.asarray(edge_index[1])
    deg = np.bincount(dst, minlength=N_NODES).astype(np.float32) + 1.0
    dinv = 1.0 / np.sqrt(deg)
    h1 = f(x) @ f(W1)  # f32 accum of bf16 operands
    TshF = dinv[:, None] * h1
    table1 = f(TshF)
    G1 = np.zeros_like(TshF)
    np.add.at(G1, dst, table1[src])
    e1 = dinv[:, None] * (G1 + TshF) + np.asarray(b1, np.float32)
    T2F = dinv[:, None] * np.maximum(e1, 0.0)
    table2 = f(T2F)
    G2 = np.zeros_like(T2F)
    np.add.at(G2, dst, table2[src])
    vs = dinv[:, None] * (G2 + T2F)
    z = f(vs) @ f(W2) + np.asarray(b2, np.float32)
    m = z.max(1, keepdims=True)
    return z - m - np.log(np.exp(z - m).sum(1, keepdims=True))


def build_nc(meta):
    import concourse.bacc as bacc
    import concourse.tile as tile
    import concourse.mybir as mybir

    dt = mybir.dt.float32
    bf = mybir.dt.bfloat16
    Alu = mybir.AluOpType
    Act = mybir.ActivationFunctionType
    B, nblk, nch = meta["B"], meta["nblk"], meta["nch"]

    nc = bacc.Bacc(None, target_bir_lowering=False, num_swdge_queues=NQ,
                   dynamic_dma_scratch_size=32768)
    p_xT = nc.declare_dram_parameter("xT", [F0, SHP], bf, isOutput=False)
    p_idx = [nc.declare_dram_parameter(f"idx{P}", [128, nch[P] * (CHUNK // 16)],
                                       mybir.dt.int16, isOutput=False)
             for P in range(2)]
    p_dl = [nc.declare_dram_parameter(f"dl{P}", [128, nblk[P]], dt,
                                      isOutput=False) for P in range(2)]
    p_deg = nc.declare_dram_parameter("deg", [128, NT], dt, isOutput=False)
    p_W1 = nc.declare_dram_parameter("W1", [F0, F1], bf, isOutput=False)
    p_W2 = nc.declare_dram_parameter("W2", [F1, F2], bf, isOutput=False)
    p_b1 = nc.declare_dram_parameter("b1b", [128, F1], dt, isOutput=False)
    p_b2 = nc.declare_dram_parameter("b2b", [128, F2], dt, isOutput=False)
    p_iota = nc.declare_dram_parameter("iota", [128, 256], bf, isOutput=False)
    p_ident = nc.declare_dram_parameter("ident", [128, 128], dt, isOutput=False)
    p_out = nc.declare_dram_parameter("out", [128, NT * F2], dt, isOutput=True)

    cc_in = [[nc.dram_tensor(f"cc_in{li}{P}", [128, (WA, WB)[P]], bf)
              for P in range(2)] for li in range(2)]
    cc_out = [[nc.dram_tensor(f"cc_out{li}{P}", [(EA, EB)[P], 128], bf,
                              addr_space="Shared")
               for P in range(2)] for li in range(2)]

    with tile.TileContext(nc) as tc:
        with (
            tc.tile_pool(name="cpool", bufs=1) as cpool,
            tc.tile_pool(name="spool", bufs=16) as spool,
            tc.tile_pool(name="stpool", bufs=24) as stpool,
            tc.tile_pool(name="wpool", bufs=4) as wpool,
            tc.tile_pool(name="ppool", bufs=4, space="PSUM") as ppool,
            tc.tile_pool(name="p2pool", bufs=2, space="PSUM") as p2pool,
        ):
            # ---- constants into SBUF
            xT = cpool.tile([F0, SHP], bf)
            nc.sync.dma_start(xT[:], p_xT[:])
            W1 = cpool.tile([F0, F1], bf)
            nc.sync.dma_start(W1[:], p_W1[:])
            W2 = cpool.tile([F1, F2], bf)
            nc.sync.dma_start(W2[:], p_W2[:])
            b1b = cpool.tile([128, F1], dt)
            nc.sync.dma_start(b1b[:], p_b1[:])
            b2b = cpool.tile([128, F2], dt)
            nc.sync.dma_start(b2b[:], p_b2[:])
            iota = cpool.tile([128, 256], bf)
            nc.sync.dma_start(iota[:], p_iota[:])
            ident = cpool.tile([128, 128], dt)
            nc.sync.dma_start(ident[:], p_ident[:])
            degt = cpool.tile([128, NT], dt)
            nc.sync.dma_start(degt[:], p_deg[:])
            idx_sb = []
            dl_sb = []
            for P in range(2):
                isb = cpool.tile([128, nch[P] * (CHUNK // 16)], mybir.dt.int16,
                                 name=f"isb{P}")
                nc.sync.dma_start(isb[:], p_idx[P][:])
                idx_sb.append(isb)
                dsb = cpool.tile([128, nblk[P]], dt, name=f"dsb{P}")
                nc.sync.dma_start(dsb[:], p_dl[P][:])
                dl_sb.append(dsb)

            recd = cpool.tile([128, NT], dt)
            nc.vector.reciprocal(recd[:], degt[:])
            dinv = cpool.tile([128, NT], dt)
            nc.scalar.activation(dinv[:], recd[:], Act.Sqrt)

            TshF = cpool.tile([128, NT * F1], dt)
            Tpad = cpool.tile([128, NT * F1], bf)
            T2F = cpool.tile([128, NT * F1], dt)
            T2pad = cpool.tile([128, NT * F1], bf)
            accA = cpool.tile([128, NT * F1], dt)
            outsh = cpool.tile([128, NT * F2], dt)

            def fire_ag(li, P, pad_src):
                w0 = 0 if P == 0 else WA
                w1 = WA if P == 0 else WA + WB
                nc.sync.dma_start(cc_in[li][P][:], pad_src[:, w0:w1])
                nc.gpsimd.collective_compute(
                    "AllGather", Alu.bypass,
                    ins=[cc_in[li][P].ap().opt()],
                    outs=[cc_out[li][P].ap().opt()],
                    replica_groups=[list(range(CORES))])

            # ---- head: TshF = dinv * (x @ W1); Tpad gets bf16 copy
            for t in range(NT):
                psh = ppool.tile([128, F1], dt, tag="agg", name=f"psh{t}")
                nc.tensor.matmul(psh[:], xT[:, BLK * t:BLK * (t + 1)], W1[:],
                                 start=True, stop=True)
                nc.vector.tensor_scalar(
                    TshF[:, F1 * t:F1 * (t + 1)], psh[:], dinv[:, t:t + 1],
                    None, Alu.mult)
                nc.scalar.copy(Tpad[:, F1 * t:F1 * (t + 1)],
                               TshF[:, F1 * t:F1 * (t + 1)])
                if t == TA - 1:
                    fire_ag(0, 0, Tpad)
            fire_ag(0, 1, Tpad)

            qcounter = [0]

            def do_pass(li, P, tail_fn):
                table = cc_out[li][P]
                emitted = [0]
                chunks = {}

                def ensure_chunk(c):
                    while emitted[0] <= min(c + LOOKAHEAD, nch[P] - 1):
                        ce = emitted[0]
                        st = stpool.tile([128, CB, 128], bf, tag="st",
                                         name=f"st_l{li}p{P}c{ce}")
                        cols = CHUNK // 16
                        nc.gpsimd.dma_gather(
                            st[:], table[:],
                            idx_sb[P][:, ce * cols:(ce + 1) * cols],
                            CHUNK, CHUNK, 128,
                            single_packet=True,
                            queue_num=qcounter[0] % NQ)
                        qcounter[0] += 1
                        chunks[ce] = st
                        if ce >= LOOKAHEAD + 2:
                            chunks.pop(ce - LOOKAHEAD - 2, None)
                        emitted[0] += 1
                    return chunks[c]

                gb = 0
                DELAY = 3
                pending = []
                for t in range(NT):
                    nb = int(B[P, t])
                    pagg = ppool.tile([128, F1], dt, tag="agg",
                                      name=f"pg{li}{P}_{t}")
                    for b in range(nb):
                        c, slot = gb // CB, gb % CB
                        st = ensure_chunk(c)
                        S = spool.tile([128, 256], bf, tag="S",
                                       name=f"S{li}{P}_{gb}")
                        nc.vector.tensor_scalar(
                            S[:], iota[:], dl_sb[P][:, gb:gb + 1], None,
                            Alu.is_equal)
                        nc.tensor.matmul(pagg[:], S[:, 0:128],
                                         st[:, slot, 0:F1],
                                         start=(b == 0), stop=False)
                        nc.tensor.matmul(pagg[:], S[:, 128:256],
                                         st[:, slot, F1:BLK],
                                         start=False, stop=(b == nb - 1))
                        gb += 1
                    pending.append((t, pagg))
                    if len(pending) > DELAY:
                        tt, pg = pending.pop(0)
                        tail_fn(tt, pg)
                for tt, pg in pending:
                    tail_fn(tt, pg)

            # ---- layer 1 pass A: spill
            def spillA(t, pagg):
                nc.scalar.copy(accA[:, F1 * t:F1 * (t + 1)], pagg[:])

            do_pass(0, 0, spillA)

            # ---- layer 1 pass B: tail computes T2
            def tail1(t, pagg):
                u = wpool.tile([128, F1], dt, tag="u", name=f"u1_{t}")
                nc.vector.tensor_tensor(
                    out=u[:], in0=pagg[:], in1=accA[:, F1 * t:F1 * (t + 1)],
                    op=Alu.add)
                v = wpool.tile([128, F1], dt, tag="v", name=f"v1_{t}")
                nc.vector.tensor_tensor(
                    out=v[:], in0=u[:], in1=TshF[:, F1 * t:F1 * (t + 1)],
                    op=Alu.add)
                e1 = wpool.tile([128, F1], dt, tag="e1", name=f"e1_{t}")
                nc.vector.scalar_tensor_tensor(
                    e1[:], v[:], dinv[:, t:t + 1], b1b[:],
                    Alu.mult, Alu.add)
                nc.vector.tensor_scalar(
                    T2F[:, F1 * t:F1 * (t + 1)], e1[:], 0.0, dinv[:, t:t + 1],
                    Alu.max, Alu.mult)
                nc.scalar.copy(T2pad[:, F1 * t:F1 * (t + 1)],
                               T2F[:, F1 * t:F1 * (t + 1)])
                if t == TA - 1:
                    fire_ag(1, 0, T2pad)
                if t == NT - 1:
                    fire_ag(1, 1, T2pad)

            do_pass(0, 1, tail1)

            # ---- layer 2 pass A
            do_pass(1, 0, spillA)

            # ---- layer 2 pass B: transpose, W2, log_softmax
            def tail2(t, pagg):
                u = wpool.tile([128, F1], dt, tag="u", name=f"u2_{t}")
                nc.vector.tensor_tensor(
                    out=u[:], in0=pagg[:], in1=accA[:, F1 * t:F1 * (t + 1)],
                    op=Alu.add)
                vs = wpool.tile([128, F1], dt, tag="v", name=f"vs_{t}")
                nc.vector.scalar_tensor_tensor(
                    vs[:], u[:], 1.0, T2F[:, F1 * t:F1 * (t + 1)],
                    Alu.mult, Alu.add)
                vsc = wpool.tile([128, F1], dt, tag="vsc", name=f"vsc_{t}")
                nc.vector.tensor_scalar(
                    vsc[:], vs[:], dinv[:, t:t + 1], None, Alu.mult)
                trp = p2pool.tile([F1, 128], dt, tag="tr", name=f"tr_{t}")
                nc.tensor.transpose(trp[:], vsc[:], ident[:])
                zT = wpool.tile([F1, 128], bf, tag="zT", name=f"zT_{t}")
                nc.scalar.copy(zT[:], trp[:])
                po = p2pool.tile([128, F2], dt, tag="po", name=f"po_{t}")
                nc.tensor.matmul(po[:], zT[:], W2[:], start=True, stop=True)
                e4 = wpool.tile([128, F2], dt, tag="e4", name=f"e4_{t}")
                nc.vector.tensor_tensor(out=e4[:], in0=po[:], in1=b2b[:],
                                        op=Alu.add)
                m = wpool.tile([128, 1], dt, tag="m", name=f"m_{t}")
                nc.vector.tensor_reduce(m[:], e4[:], axis=mybir.AxisListType.X,
                                        op=Alu.max)
                nm = wpool.tile([128, 1], dt, tag="nm", name=f"nm_{t}")
                nc.vector.tensor_scalar(nm[:], m[:], -1.0, None, Alu.mult)
                ex = wpool.tile([128, F2], dt, tag="ex", name=f"ex_{t}")
                nc.scalar.activation(ex[:], e4[:], Act.Exp, bias=nm[:, 0:1])
                sm = wpool.tile([128, 1], dt, tag="sm", name=f"sm_{t}")
                nc.vector.tensor_reduce(sm[:], ex[:], axis=mybir.AxisListType.X,
                                        op=Alu.add)
                lg = wpool.tile([128, 1], dt, tag="lg", name=f"lg_{t}")
                nc.scalar.activation(lg[:], sm[:], Act.Ln)
                nc.vector.tensor_scalar(
                    outsh[:, F2 * t:F2 * (t + 1)], e4[:], m[:, 0:1],
                    lg[:, 0:1], Alu.subtract, Alu.subtract)

            do_pass(1, 1, tail2)
            nc.sync.dma_start(p_out[:], outsh[:])

    nc.finalize()
    return nc


LAST_EXEC_NS = None


def kernel(x, edge_index, W1, b1, W2, b2):
    from concourse.bass_utils import run_bass_kernel_spmd

    x = np.asarray(x, np.float32)
    data, consts, meta = host_prep(x, np.asarray(edge_index), W1, b1, W2, b2)
    nc = build_nc(meta)
    in_maps = []
    for i in range(CORES):
        m = dict(data[i])
        m.update({k: np.ascontiguousarray(v) for k, v in consts.items()})
        in_maps.append(m)
    import os as _os
    trace = bool(int(_os.environ.get("GCN_TRACE", "0")))
    res = run_bass_kernel_spmd(nc, in_maps, core_ids=list(range(CORES)),
                               trace=trace)
    global LAST_EXEC_NS
    LAST_EXEC_NS = res.exec_time_ns
    if trace and res.instructions_and_trace:
        try:
            import pickle
            insts, tpath = res.instructions_and_trace
            with open("/tmp/gcn_insts.pkl", "wb") as f:
                pickle.dump({"insts": insts, "exec_ns": res.exec_time_ns,
                             "trace_path": tpath}, f)
        except Exception as e:
            print("trace stash failed:", e)
    outs = []
    for i in range(CORES):
        o = res.results[i]["out"]  # [128, NT*F2]
        outs.append(o.reshape(128, NT, F2).transpose(1, 0, 2).reshape(SHP, F2))
    res_full = np.zeros((N_NODES, F2), np.float32)
    for i in range(CORES):
        res_full[SH * i:SH * (i + 1)] = outs[i][:SH]
    return res_full


def replay_check(inputs, data, meta, core=3):
    """Replay core `core`'s layer-1 streams against a direct edge sum."""
    x, ei = inputs["x"], inputs["edge_index"]
    W1 = inputs["W1"]
    src, dst = np.asarray(ei[0]), np.asarray(ei[1])
    deg = np.bincount(dst, minlength=N_NODES).astype(np.float32) + 1.0
    dinv = 1.0 / np.sqrt(deg)
    h1 = _bf(x).astype(np.float32) @ _bf(W1).astype(np.float32)
    TshF = dinv[:, None] * h1
    tb = _bf(TshF).astype(np.float32)
    # padded-row table per shard [128, WA] / [128, WB]
    ccA = np.zeros((CORES, BLK, WA), np.float32)
    ccB = np.zeros((CORES, BLK, WB), np.float32)
    for s in range(CORES):
        sh = np.zeros((SHP, F1), np.float32)
        sh[:SH] = tb[SH * s:SH * (s + 1)]
        g = sh.reshape(NT, BLK, F1)
        ccA[s] = g[:TA].transpose(1, 0, 2).reshape(BLK, WA)
        ccB[s] = g[TA:].transpose(1, 0, 2).reshape(BLK, WB)
    elemsA = ccA.reshape(-1)  # flat bf16-unit stream
    elemsB = ccB.reshape(-1)
    EAr = elemsA.reshape(EA, BLK)
    EBr = elemsB.reshape(EB, BLK)

    d = data[core]
    B = meta["B"]
    agg = np.zeros((NT, BLK, F1), np.float32)
    for P in range(2):
        tab = EAr if P == 0 else EBr
        idxp = d[f"idx{P}"]
        stream = idxp[:16].T.reshape(-1).astype(np.int64)
        dlp = d[f"dl{P}"]
        gb = 0
        for t in range(NT):
            for b in range(int(B[P, t])):
                rows = stream[gb * BLK:(gb + 1) * BLK]
                G = tab[rows]                      # [128, 128]
                dl = dlp[:, gb]
                for e in range(BLK):
                    dv = int(dl[e])
                    if dv >= 2 * BLK:
                        continue
                    par, dd = dv // BLK, dv % BLK
                    agg[t, dd] += G[e, F1 * par:F1 * par + F1]
                gb += 1
    # ground truth for this core's shard
    G1 = np.zeros((N_NODES, F1), np.float32)
    np.add.at(G1, dst, tb[src])
    gt = np.zeros((SHP, F1), np.float32)
    gt[:SH] = G1[SH * core:SH * (core + 1)]
    got = agg.reshape(SHP, F1)
    err = np.abs(got - gt).max()
    print(f"replay: max abs err {err:.3e} (scale {np.abs(gt).max():.2f})")
    assert err < 2e-2, "stream replay mismatch"


if __name__ == "__main__":
    z = np.load("/tmp/gcn_ref.npz")
    inputs = {k: z[k] for k in z.files if k != "expected"}
    expected = z["expected"]
    data, consts, meta = host_prep(**inputs)
    print("nblk:", meta["nblk"], "nch:", meta["nch"])
    got = numpy_sim(**inputs)
    err = np.abs(got - expected)
    rel = err.max() / np.abs(expected).max()
    print(f"numpy-sim (bf16 emul) max abs err {err.max():.3e}  rel {rel:.3e}")
    replay_check(inputs, data, meta)


# revision 13
# speedup vs baseline: 1.0640x; 1.0640x over previous
"""GCN (2-layer) Trainium2 kernel over 8 NeuronCores — v2.

Strategy (dst-sharded pull-gather, bf16):
- Nodes sharded contiguously: core i owns nodes [6250*i, 6250*(i+1)).
- Layer table rows are bf16, PADDED to 128 values (64 real + 64 unread) so
  every row is one 256-byte gather element (dma_gather requires 256B-aligned
  elements; gather cost is per-descriptor latency, so padding is free).
- The table is AllGather'd in two halves (src tiles 0:25 / 25:49) so the
  second half's collective overlaps the first half's gather+aggregate pass.
- Aggregation per 128-dst tile: gathered 128-edge blocks are summed with a
  TensorE matmul against a DVE-built one-hot selector S (bf16).
- Self-loop terms are folded into the per-tile tail math (never gathered).
- Gathers run in 2048-index chunks round-robined over 4 SWDGE queues.
- Layer 2 aggregates the (dinv*relu(.)) table, then applies W2 after a PE
  transpose; log_softmax on ACT+DVE.
The edge structure is baked into the program; block counts are padded to the
max over cores so the SPMD program is identical on all 8 cores.
"""

import numpy as np

try:
    from ml_dtypes import bfloat16 as bf16np
except ImportError:  # pragma: no cover
    bf16np = None

N_NODES = 50000
CORES = 8
SH = 6250          # owned nodes per core
SHP = 6272         # padded shard rows (49*128)
NT = 49            # dst tiles per core
BLK = 128
F0, F1, F2 = 96, 64, 16
TA = 25            # tiles in half A
TB = NT - TA       # 24
WA, WB = TA * F1, TB * F1            # compact cols per half: 1600 / 1536
EA, EB = 1024 * WA // BLK, 1024 * WB // BLK  # 256B elements per half: 12800 / 12288
CHUNK = 1024
CB = CHUNK // BLK  # blocks per chunk
NQ = 4             # SWDGE queues
PAD_DL = 300.0     # is_equal miss => zero S row
LOOKAHEAD = 20


def _bf(x):
    return np.asarray(x, np.float32).astype(bf16np)


def host_prep(x, edge_index, W1, b1, W2, b2):
    src = np.asarray(edge_index[0], dtype=np.int64)
    dst = np.asarray(edge_index[1], dtype=np.int64)
    deg_full = np.bincount(dst, minlength=N_NODES).astype(np.float32) + 1.0

    # split edges by dst shard
    order = np.argsort(dst, kind="stable")
    s_sorted, d_sorted = src[order], dst[order]
    bounds = np.searchsorted(d_sorted, np.arange(0, N_NODES + 1, SH))

    # per-core, per-pass, per-tile edge lists: (elem, dl)
    counts = np.zeros((CORES, 2, NT), np.int64)
    lists = [[[None] * NT for _ in range(2)] for _ in range(CORES)]
    for i in range(CORES):
        es = s_sorted[bounds[i]:bounds[i + 1]]
        ed = d_sorted[bounds[i]:bounds[i + 1]] - SH * i
        s_sh = es // SH
        l = es - SH * s_sh
        ts = l // BLK
        p = l - ts * BLK
        half = (ts >= TA).astype(np.int64)
        rowA = WA * (s_sh * BLK + p) + F1 * ts          # bf16 units
        rowB = WB * (s_sh * BLK + p) + F1 * (ts - TA)
        elem = np.where(half == 0, rowA // BLK, rowB // BLK)
        par = np.where(half == 0, (rowA // F1) % 2, (rowB // F1) % 2)
        t = ed // BLK
        dl = ed - t * BLK + BLK * par                    # 0..255
        key = half * NT + t
        o = np.argsort(key, kind="stable")
        key_s, elem_s, dl_s = key[o], elem[o], dl[o]
        kb = np.searchsorted(key_s, np.arange(2 * NT + 1))
        for P in range(2):
            for tt in range(NT):
                a, b = kb[P * NT + tt], kb[P * NT + tt + 1]
                lists[i][P][tt] = (elem_s[a:b], dl_s[a:b])
                counts[i, P, tt] = b - a

    # uniform block counts across cores
    B = np.maximum(1, -(-counts.max(axis=0) // BLK))  # [2, NT]
    nblk = [int(B[P].sum()) for P in range(2)]
    nch = [-(-nblk[P] // CB) for P in range(2)]

    data = []
    for i in range(CORES):
        d = dict()
        for P in range(2):
            els, dls = [], []
            for tt in range(NT):
                e, q = lists[i][P][tt]
                pad = int(B[P, tt]) * BLK - len(e)
                els.append(np.concatenate([e, np.zeros(pad, np.int64)]))
                dls.append(np.concatenate([q.astype(np.float32),
                                           np.full(pad, PAD_DL, np.float32)]))
            estream = np.concatenate(els)
            dstream = np.concatenate(dls)
            tail = nch[P] * CHUNK - len(estream)
            estream = np.concatenate([estream, np.zeros(tail, np.int64)])
            # idx plane [128, nch*128]: idx j at [j%16, j//16], replicated x8
            pl = estream.reshape(-1, 16).T.astype(np.int16)
            d[f"idx{P}"] = np.ascontiguousarray(np.tile(pl, (8, 1)))
            # dl plane [128, nblk]
            d[f"dl{P}"] = np.ascontiguousarray(
                dstream.reshape(-1, BLK).T.astype(np.float32))
        degp = np.ones((BLK, NT), np.float32)
        dsh = deg_full[SH * i:SH * (i + 1)]
        dp = np.concatenate([dsh, np.ones(SHP - SH, np.float32)])
        degp[:, :] = dp.reshape(NT, BLK).T
        d["deg"] = np.ascontiguousarray(degp)
        xs = np.zeros((F0, SHP), np.float32)
        xs[:, :SH] = np.asarray(x[SH * i:SH * (i + 1)], np.float32).T
        d["xT"] = np.ascontiguousarray(_bf(xs))
        data.append(d)

    ident = np.eye(BLK, dtype=np.float32)
    consts = dict(
        W1=_bf(W1), W2=_bf(W2),
        b1b=np.tile(np.asarray(b1, np.float32), (BLK, 1)),
        b2b=np.tile(np.asarray(b2, np.float32), (BLK, 1)),
        iota=_bf(np.tile(np.arange(2 * BLK, dtype=np.float32), (BLK, 1))),
        ident=ident,
    )
    meta = dict(B=B, nblk=nblk, nch=nch)
    return data, consts, meta


def numpy_sim(x, edge_index, W1, b1, W2, b2):
    """Emulate the device numerics (bf16 tables/weights) edge-wise."""
    def f(a):
        return _bf(a).astype(np.float32)

    src = np.asarray(edge_index[0]); dst = np.asarray(edge_index[1])
    deg = np.bincount(dst, minlength=N_NODES).astype(np.float32) + 1.0
    dinv = 1.0 / np.sqrt(deg)
    h1 = f(x) @ f(W1)  # f32 accum of bf16 operands
    TshF = dinv[:, None] * h1
    table1 = f(TshF)
    G1 = np.zeros_like(TshF)
    np.add.at(G1, dst, table1[src])
    e1 = dinv[:, None] * (G1 + TshF) + np.asarray(b1, np.float32)
    T2F = dinv[:, None] * np.maximum(e1, 0.0)
    table2 = f(T2F)
    G2 = np.zeros_like(T2F)
    np.add.at(G2, dst, table2[src])
    vs = dinv[:, None] * (G2 + T2F)
    z = f(vs) @ f(W2) + np.asarray(b2, np.float32)
    m = z.max(1, keepdims=True)
    return z - m - np.log(np.exp(z - m).sum(1, keepdims=True))


def build_nc(meta):
    import concourse.bacc as bacc
    import concourse.tile as tile
    import concourse.mybir as mybir

    dt = mybir.dt.float32
    bf = mybir.dt.bfloat16
    Alu = mybir.AluOpType
    Act = mybir.ActivationFunctionType
    B, nblk, nch = meta["B"], meta["nblk"], meta["nch"]

    nc = bacc.Bacc(None, target_bir_lowering=False, num_swdge_queues=NQ,
                   dynamic_dma_scratch_size=32768)
    p_xT = nc.declare_dram_parameter("xT", [F0, SHP], bf, isOutput=False)
    p_idx = [nc.declare_dram_parameter(f"idx{P}", [128, nch[P] * (CHUNK // 16)],
                                       mybir.dt.int16, isOutput=False)
             for P in range(2)]
    p_dl = [nc.declare_dram_parameter(f"dl{P}", [128, nblk[P]], dt,
                                      isOutput=False) for P in range(2)]
    p_deg = nc.declare_dram_parameter("deg", [128, NT], dt, isOutput=False)
    p_W1 = nc.declare_dram_parameter("W1", [F0, F1], bf, isOutput=False)
    p_W2 = nc.declare_dram_parameter("W2", [F1, F2], bf, isOutput=False)
    p_b1 = nc.declare_dram_parameter("b1b", [128, F1], dt, isOutput=False)
    p_b2 = nc.declare_dram_parameter("b2b", [128, F2], dt, isOutput=False)
    p_iota = nc.declare_dram_parameter("iota", [128, 256], bf, isOutput=False)
    p_ident = nc.declare_dram_parameter("ident", [128, 128], dt, isOutput=False)
    p_out = nc.declare_dram_parameter("out", [128, NT * F2], dt, isOutput=True)

    cc_in = [[nc.dram_tensor(f"cc_in{li}{P}", [128, (WA, WB)[P]], bf)
              for P in range(2)] for li in range(2)]
    cc_out = [[nc.dram_tensor(f"cc_out{li}{P}", [(EA, EB)[P], 128], bf,
                              addr_space="Shared")
               for P in range(2)] for li in range(2)]

    with tile.TileContext(nc) as tc:
        with (
            tc.tile_pool(name="cpool", bufs=1) as cpool,
            tc.tile_pool(name="spool", bufs=16) as spool,
            tc.tile_pool(name="stpool", bufs=24) as stpool,
            tc.tile_pool(name="wpool", bufs=4) as wpool,
            tc.tile_pool(name="ppool", bufs=4, space="PSUM") as ppool,
            tc.tile_pool(name="p2pool", bufs=2, space="PSUM") as p2pool,
        ):
            # ---- constants into SBUF
            xT = cpool.tile([F0, SHP], bf)
            nc.sync.dma_start(xT[:], p_xT[:])
            W1 = cpool.tile([F0, F1], bf)
            nc.sync.dma_start(W1[:], p_W1[:])
            W2 = cpool.tile([F1, F2], bf)
            nc.sync.dma_start(W2[:], p_W2[:])
            b1b = cpool.tile([128, F1], dt)
            nc.sync.dma_start(b1b[:], p_b1[:])
            b2b = cpool.tile([128, F2], dt)
            nc.sync.dma_start(b2b[:], p_b2[:])
            iota = cpool.tile([128, 256], bf)
            nc.sync.dma_start(iota[:], p_iota[:])
            ident = cpool.tile([128, 128], dt)
            nc.sync.dma_start(ident[:], p_ident[:])
            degt = cpool.tile([128, NT], dt)
            nc.sync.dma_start(degt[:], p_deg[:])
            idx_sb = []
            dl_sb = []
            for P in range(2):
                isb = cpool.tile([128, nch[P] * (CHUNK // 16)], mybir.dt.int16,
                                 name=f"isb{P}")
                nc.sync.dma_start(isb[:], p_idx[P][:])
                idx_sb.append(isb)
                dsb = cpool.tile([128, nblk[P]], dt, name=f"dsb{P}")
                nc.sync.dma_start(dsb[:], p_dl[P][:])
                dl_sb.append(dsb)

            recd = cpool.tile([128, NT], dt)
            nc.vector.reciprocal(recd[:], degt[:])
            dinv = cpool.tile([128, NT], dt)
            nc.scalar.activation(dinv[:], recd[:], Act.Sqrt)

            TshF = cpool.tile([128, NT * F1], dt)
            Tpad = cpool.tile([128, NT * F1], bf)
            T2F = cpool.tile([128, NT * F1], dt)
            T2pad = cpool.tile([128, NT * F1], bf)
            accA = cpool.tile([128, NT * F1], dt)
            accB = cpool.tile([128, NT * F1], dt)
            outsh = cpool.tile([128, NT * F2], dt)

            def fire_ag(li, P, pad_src):
                w0 = 0 if P == 0 else WA
                w1 = WA if P == 0 else WA + WB
                nc.sync.dma_start(cc_in[li][P][:], pad_src[:, w0:w1])
                nc.gpsimd.collective_compute(
                    "AllGather", Alu.bypass,
                    ins=[cc_in[li][P].ap().opt()],
                    outs=[cc_out[li][P].ap().opt()],
                    replica_groups=[list(range(CORES))])

            # ---- head: TshF = dinv * (x @ W1); Tpad gets bf16 copy
            for t in range(NT):
                psh = ppool.tile([128, F1], dt, tag="agg", name=f"psh{t}")
                nc.tensor.matmul(psh[:], xT[:, BLK * t:BLK * (t + 1)], W1[:],
                                 start=True, stop=True)
                nc.vector.tensor_scalar(
                    TshF[:, F1 * t:F1 * (t + 1)], psh[:], dinv[:, t:t + 1],
                    None, Alu.mult)
                nc.scalar.copy(Tpad[:, F1 * t:F1 * (t + 1)],
                               TshF[:, F1 * t:F1 * (t + 1)])
                if t == TA - 1:
                    fire_ag(0, 0, Tpad)
            fire_ag(0, 1, Tpad)

            qcounter = [0]

            def do_pass(li, P, tail_fn):
                table = cc_out[li][P]
                emitted = [0]
                chunks = {}

                def ensure_chunk(c):
                    while emitted[0] <= min(c + LOOKAHEAD, nch[P] - 1):
                        ce = emitted[0]
                        st = stpool.tile([128, CB, 128], bf, tag="st",
                                         name=f"st_l{li}p{P}c{ce}")
                        cols = CHUNK // 16
                        nc.gpsimd.dma_gather(
                            st[:], table[:],
                            idx_sb[P][:, ce * cols:(ce + 1) * cols],
                            CHUNK, CHUNK, 128,
                            single_packet=True,
                            queue_num=qcounter[0] % NQ)
                        qcounter[0] += 1
                        chunks[ce] = st
                        if ce >= LOOKAHEAD + 2:
                            chunks.pop(ce - LOOKAHEAD - 2, None)
                        emitted[0] += 1
                    return chunks[c]

                gb = 0
                for t in range(NT):
                    nb = int(B[P, t])
                    pagg = ppool.tile([128, F1], dt, tag="agg",
                                      name=f"pg{li}{P}_{t}")
                    for b in range(nb):
                        c, slot = gb // CB, gb % CB
                        st = ensure_chunk(c)
                        S = spool.tile([128, 256], bf, tag="S",
                                       name=f"S{li}{P}_{gb}")
                        nc.vector.tensor_scalar(
                            S[:], iota[:], dl_sb[P][:, gb:gb + 1], None,
                            Alu.is_equal)
                        nc.tensor.matmul(pagg[:], S[:, 0:128],
                                         st[:, slot, 0:F1],
                                         start=(b == 0), stop=False)
                        nc.tensor.matmul(pagg[:], S[:, 128:256],
                                         st[:, slot, F1:BLK],
                                         start=False, stop=(b == nb - 1))
                        gb += 1
                    tail_fn(t, pagg)

            # ---- layer 1 pass A: spill
            def spillA(t, pagg):
                nc.scalar.copy(accA[:, F1 * t:F1 * (t + 1)], pagg[:])

            do_pass(0, 0, spillA)

            # ---- layer 1 pass B: spill to accB, tails in a post-loop
            def spillB(t, pagg):
                nc.scalar.copy(accB[:, F1 * t:F1 * (t + 1)], pagg[:])

            do_pass(0, 1, spillB)

            for t in range(NT):
                u = wpool.tile([128, F1], dt, tag="u", name=f"u1_{t}")
                nc.vector.tensor_tensor(
                    out=u[:], in0=accB[:, F1 * t:F1 * (t + 1)],
                    in1=accA[:, F1 * t:F1 * (t + 1)], op=Alu.add)
                v = wpool.tile([128, F1], dt, tag="v", name=f"v1_{t}")
                nc.vector.tensor_tensor(
                    out=v[:], in0=u[:], in1=TshF[:, F1 * t:F1 * (t + 1)],
                    op=Alu.add)
                e1 = wpool.tile([128, F1], dt, tag="e1", name=f"e1_{t}")
                nc.vector.scalar_tensor_tensor(
                    e1[:], v[:], dinv[:, t:t + 1], b1b[:],
                    Alu.mult, Alu.add)
                nc.vector.tensor_scalar(
                    T2F[:, F1 * t:F1 * (t + 1)], e1[:], 0.0, dinv[:, t:t + 1],
                    Alu.max, Alu.mult)
                nc.scalar.copy(T2pad[:, F1 * t:F1 * (t + 1)],
                               T2F[:, F1 * t:F1 * (t + 1)])
                if t == TA - 1:
                    fire_ag(1, 0, T2pad)
                if t == NT - 1:
                    fire_ag(1, 1, T2pad)

            # ---- layer 2 pass A
            do_pass(1, 0, spillA)

            # ---- layer 2 pass B: spill, then transpose, W2, log_softmax
            do_pass(1, 1, spillB)

            def tail2(t, pagg):
                u = wpool.tile([128, F1], dt, tag="u", name=f"u2_{t}")
                nc.vector.tensor_tensor(
                    out=u[:], in0=accB[:, F1 * t:F1 * (t + 1)],
                    in1=accA[:, F1 * t:F1 * (t + 1)], op=Alu.add)
                vs = wpool.tile([128, F1], dt, tag="v", name=f"vs_{t}")
                nc.vector.scalar_tensor_tensor(
                    vs[:], u[:], 1.0, T2F[:, F1 * t:F1 * (t + 1)],
                    Alu.mult, Alu.add)
                vsc = wpool.tile([128, F1], dt, tag="vsc", name=f"vsc_{t}")
                nc.vector.tensor_scalar(
                    vsc[:], vs[:], dinv[:, t:t + 1], None, Alu.mult)
                trp = p2pool.tile([F1, 128], dt, tag="tr", name=f"tr_{t}")
                nc.tensor.transpose(trp[:], vsc[:], ident[:])
                zT = wpool.tile([F1, 128], bf, tag="zT", name=f"zT_{t}")
                nc.scalar.copy(zT[:], trp[:])
                po = p2pool.tile([128, F2], dt, tag="po", name=f"po_{t}")
                nc.tensor.matmul(po[:], zT[:], W2[:], start=True, stop=True)
                e4 = wpool.tile([128, F2], dt, tag="e4", name=f"e4_{t}")
                nc.vector.tensor_tensor(out=e4[:], in0=po[:], in1=b2b[:],
                                        op=Alu.add)
                m = wpool.tile([128, 1], dt, tag="m", name=f"m_{t}")
                nc.vector.tensor_reduce(m[:], e4[:], axis=mybir.AxisListType.X,
                                        op=Alu.max)
                nm = wpool.tile([128, 1], dt, tag="nm", name=f"nm_{t}")
                nc.vector.tensor_scalar(nm[:], m[:], -1.0, None, Alu.mult)
                ex = wpool.tile([128, F2], dt, tag="ex", name=f"ex_{t}")
                nc.scalar.activation(ex[:], e4[:], Act.Exp, bias=nm[:, 0:1])
                sm = wpool.tile([128, 1], dt, tag="sm", name=f"sm_{t}")
                nc.vector.tensor_reduce(sm[:], ex[:], axis=mybir.AxisListType.X,
                                        op=Alu.add)
                lg = wpool.tile([128, 1], dt, tag="lg", name=f"lg_{t}")
                nc.scalar.activation(lg[:], sm[:], Act.Ln)
                nc.vector.tensor_scalar(
                    outsh[:, F2 * t:F2 * (t + 1)], e4[:], m[:, 0:1],
                    lg[:, 0:1], Alu.subtract, Alu.subtract)

            for t in range(NT):
                tail2(t, None)
            nc.sync.dma_start(p_out[:], outsh[:])

    nc.finalize()
    return nc


LAST_EXEC_NS = None


def kernel(x, edge_index, W1, b1, W2, b2):
    from concourse.bass_utils import run_bass_kernel_spmd

    x = np.asarray(x, np.float32)
    data, consts, meta = host_prep(x, np.asarray(edge_index), W1, b1, W2, b2)
    nc = build_nc(meta)
    in_maps = []
    for i in range(CORES):
        m = dict(data[i])
        m.update({k: np.ascontiguousarray(v) for k, v in consts.items()})
        in_maps.append(m)
    import os as _os
    trace = bool(int(_os.environ.get("GCN_TRACE", "0")))
    res = run_bass_kernel_spmd(nc, in_maps, core_ids=list(range(CORES)),
                               trace=trace)
    global LAST_EXEC_NS
    LAST_EXEC_NS = res.exec_time_ns
    if trace and res.instructions_and_trace:
        try:
            import pickle
            insts, tpath = res.instructions_and_trace
            with open("/tmp/gcn_insts.pkl", "wb") as f:
                pickle.dump({"insts": insts, "exec_ns": res.exec_time_ns,
                             "trace_path": tpath}, f)
        except Exception as e:
            print("trace stash failed:", e)
    outs = []
    for i in range(CORES):
        o = res.results[i]["out"]  # [128, NT*F2]
        outs.append(o.reshape(128, NT, F2).transpose(1, 0, 2).reshape(SHP, F2))
    res_full = np.zeros((N_NODES, F2), np.float32)
    for i in range(CORES):
        res_full[SH * i:SH * (i + 1)] = outs[i][:SH]
    return res_full


def replay_check(inputs, data, meta, core=3):
    """Replay core `core`'s layer-1 streams against a direct edge sum."""
    x, ei = inputs["x"], inputs["edge_index"]
    W1 = inputs["W1"]
    src, dst = np.asarray(ei[0]), np.asarray(ei[1])
    deg = np.bincount(dst, minlength=N_NODES).astype(np.float32) + 1.0
    dinv = 1.0 / np.sqrt(deg)
    h1 = _bf(x).astype(np.float32) @ _bf(W1).astype(np.float32)
    TshF = dinv[:, None] * h1
    tb = _bf(TshF).astype(np.float32)
    # padded-row table per shard [128, WA] / [128, WB]
    ccA = np.zeros((CORES, BLK, WA), np.float32)
    ccB = np.zeros((CORES, BLK, WB), np.float32)
    for s in range(CORES):
        sh = np.zeros((SHP, F1), np.float32)
        sh[:SH] = tb[SH * s:SH * (s + 1)]
        g = sh.reshape(NT, BLK, F1)
        ccA[s] = g[:TA].transpose(1, 0, 2).reshape(BLK, WA)
        ccB[s] = g[TA:].transpose(1, 0, 2).reshape(BLK, WB)
    elemsA = ccA.reshape(-1)  # flat bf16-unit stream
    elemsB = ccB.reshape(-1)
    EAr = elemsA.reshape(EA, BLK)
    EBr = elemsB.reshape(EB, BLK)

    d = data[core]
    B = meta["B"]
    agg = np.zeros((NT, BLK, F1), np.float32)
    for P in range(2):
        tab = EAr if P == 0 else EBr
        idxp = d[f"idx{P}"]
        stream = idxp[:16].T.reshape(-1).astype(np.int64)
        dlp = d[f"dl{P}"]
        gb = 0
        for t in range(NT):
            for b in range(int(B[P, t])):
                rows = stream[gb * BLK:(gb + 1) * BLK]
                G = tab[rows]                      # [128, 128]
                dl = dlp[:, gb]
                for e in range(BLK):
                    dv = int(dl[e])
                    if dv >= 2 * BLK:
                        continue
                    par, dd = dv // BLK, dv % BLK
                    agg[t, dd] += G[e, F1 * par:F1 * par + F1]
                gb += 1
    # ground truth for this core's shard
    G1 = np.zeros((N_NODES, F1), np.float32)
    np.add.at(G1, dst, tb[src])
    gt = np.zeros((SHP, F1), np.float32)
    gt[:SH] = G1[SH * core:SH * (core + 1)]
    got = agg.reshape(SHP, F1)
    err = np.abs(got - gt).max()
    print(f"replay: max abs err {err:.3e} (scale {np.abs(gt).max():.2f})")
    assert err < 2e-2, "stream replay mismatch"


if __name__ == "__main__":
    z = np.load("/tmp/gcn_ref.npz")
    inputs = {k: z[k] for k in z.files if k != "expected"}
    expected = z["expected"]
    data, consts, meta = host_prep(**inputs)
    print("nblk:", meta["nblk"], "nch:", meta["nch"])
    got = numpy_sim(**inputs)
    err = np.abs(got - expected)
    rel = err.max() / np.abs(expected).max()
    print(f"numpy-sim (bf16 emul) max abs err {err.max():.3e}  rel {rel:.3e}")
    replay_check(inputs, data, meta)


# revision 14
# speedup vs baseline: 1.0709x; 1.0065x over previous
"""GCN (2-layer) Trainium2 kernel over 8 NeuronCores — v2.

Strategy (dst-sharded pull-gather, bf16):
- Nodes sharded contiguously: core i owns nodes [6250*i, 6250*(i+1)).
- Layer table rows are bf16, PADDED to 128 values (64 real + 64 unread) so
  every row is one 256-byte gather element (dma_gather requires 256B-aligned
  elements; gather cost is per-descriptor latency, so padding is free).
- The table is AllGather'd in two halves (src tiles 0:25 / 25:49) so the
  second half's collective overlaps the first half's gather+aggregate pass.
- Aggregation per 128-dst tile: gathered 128-edge blocks are summed with a
  TensorE matmul against a DVE-built one-hot selector S (bf16).
- Self-loop terms are folded into the per-tile tail math (never gathered).
- Gathers run in 2048-index chunks round-robined over 4 SWDGE queues.
- Layer 2 aggregates the (dinv*relu(.)) table, then applies W2 after a PE
  transpose; log_softmax on ACT+DVE.
The edge structure is baked into the program; block counts are padded to the
max over cores so the SPMD program is identical on all 8 cores.
"""

import numpy as np

try:
    from ml_dtypes import bfloat16 as bf16np
except ImportError:  # pragma: no cover
    bf16np = None

N_NODES = 50000
CORES = 8
SH = 6250          # owned nodes per core
SHP = 6272         # padded shard rows (49*128)
NT = 49            # dst tiles per core
BLK = 128
F0, F1, F2 = 96, 64, 16
TA = 25            # tiles in half A
TB = NT - TA       # 24
WA, WB = TA * F1, TB * F1            # compact cols per half: 1600 / 1536
EA, EB = 1024 * WA // BLK, 1024 * WB // BLK  # 256B elements per half: 12800 / 12288
CHUNK = 1024
CB = CHUNK // BLK  # blocks per chunk
NQ = 4             # SWDGE queues
PAD_DL = 300.0     # is_equal miss => zero S row
LOOKAHEAD = 20


def _bf(x):
    return np.asarray(x, np.float32).astype(bf16np)


def host_prep(x, edge_index, W1, b1, W2, b2):
    src = np.asarray(edge_index[0], dtype=np.int64)
    dst = np.asarray(edge_index[1], dtype=np.int64)
    deg_full = np.bincount(dst, minlength=N_NODES).astype(np.float32) + 1.0

    # split edges by dst shard
    order = np.argsort(dst, kind="stable")
    s_sorted, d_sorted = src[order], dst[order]
    bounds = np.searchsorted(d_sorted, np.arange(0, N_NODES + 1, SH))

    # per-core, per-pass, per-tile edge lists: (elem, dl)
    counts = np.zeros((CORES, 2, NT), np.int64)
    lists = [[[None] * NT for _ in range(2)] for _ in range(CORES)]
    for i in range(CORES):
        es = s_sorted[bounds[i]:bounds[i + 1]]
        ed = d_sorted[bounds[i]:bounds[i + 1]] - SH * i
        s_sh = es // SH
        l = es - SH * s_sh
        ts = l // BLK
        p = l - ts * BLK
        half = (ts >= TA).astype(np.int64)
        rowA = WA * (s_sh * BLK + p) + F1 * ts          # bf16 units
        rowB = WB * (s_sh * BLK + p) + F1 * (ts - TA)
        elem = np.where(half == 0, rowA // BLK, rowB // BLK)
        par = np.where(half == 0, (rowA // F1) % 2, (rowB // F1) % 2)
        t = ed // BLK
        dl = ed - t * BLK + BLK * par                    # 0..255
        key = half * NT + t
        o = np.argsort(key, kind="stable")
        key_s, elem_s, dl_s = key[o], elem[o], dl[o]
        kb = np.searchsorted(key_s, np.arange(2 * NT + 1))
        for P in range(2):
            for tt in range(NT):
                a, b = kb[P * NT + tt], kb[P * NT + tt + 1]
                lists[i][P][tt] = (elem_s[a:b], dl_s[a:b])
                counts[i, P, tt] = b - a

    # uniform block counts across cores
    B = np.maximum(1, -(-counts.max(axis=0) // BLK))  # [2, NT]
    nblk = [int(B[P].sum()) for P in range(2)]
    nch = [-(-nblk[P] // CB) for P in range(2)]

    data = []
    for i in range(CORES):
        d = dict()
        for P in range(2):
            els, dls = [], []
            for tt in range(NT):
                e, q = lists[i][P][tt]
                pad = int(B[P, tt]) * BLK - len(e)
                els.append(np.concatenate([e, np.zeros(pad, np.int64)]))
                dls.append(np.concatenate([q.astype(np.float32),
                                           np.full(pad, PAD_DL, np.float32)]))
            estream = np.concatenate(els)
            dstream = np.concatenate(dls)
            tail = nch[P] * CHUNK - len(estream)
            estream = np.concatenate([estream, np.zeros(tail, np.int64)])
            # idx plane [128, nch*128]: idx j at [j%16, j//16], replicated x8
            pl = estream.reshape(-1, 16).T.astype(np.int16)
            d[f"idx{P}"] = np.ascontiguousarray(np.tile(pl, (8, 1)))
            # dl plane [128, nblk]
            d[f"dl{P}"] = np.ascontiguousarray(
                dstream.reshape(-1, BLK).T.astype(np.float32))
        degp = np.ones((BLK, NT), np.float32)
        dsh = deg_full[SH * i:SH * (i + 1)]
        dp = np.concatenate([dsh, np.ones(SHP - SH, np.float32)])
        degp[:, :] = dp.reshape(NT, BLK).T
        d["deg"] = np.ascontiguousarray(degp)
        xs = np.zeros((F0, SHP), np.float32)
        xs[:, :SH] = np.asarray(x[SH * i:SH * (i + 1)], np.float32).T
        d["xT"] = np.ascontiguousarray(_bf(xs))
        data.append(d)

    ident = np.eye(BLK, dtype=np.float32)
    consts = dict(
        W1=_bf(W1), W2=_bf(W2),
        b1b=np.tile(np.asarray(b1, np.float32), (BLK, 1)),
        b2b=np.tile(np.asarray(b2, np.float32), (BLK, 1)),
        iota=_bf(np.tile(np.arange(2 * BLK, dtype=np.float32), (BLK, 1))),
        ident=ident,
    )
    meta = dict(B=B, nblk=nblk, nch=nch)
    return data, consts, meta


def numpy_sim(x, edge_index, W1, b1, W2, b2):
    """Emulate the device numerics (bf16 tables/weights) edge-wise."""
    def f(a):
        return _bf(a).astype(np.float32)

    src = np.asarray(edge_index[0]); dst = np.asarray(edge_index[1])
    deg = np.bincount(dst, minlength=N_NODES).astype(np.float32) + 1.0
    dinv = 1.0 / np.sqrt(deg)
    h1 = f(x) @ f(W1)  # f32 accum of bf16 operands
    TshF = dinv[:, None] * h1
    table1 = f(TshF)
    G1 = np.zeros_like(TshF)
    np.add.at(G1, dst, table1[src])
    e1 = dinv[:, None] * (G1 + TshF) + np.asarray(b1, np.float32)
    T2F = dinv[:, None] * np.maximum(e1, 0.0)
    table2 = f(T2F)
    G2 = np.zeros_like(T2F)
    np.add.at(G2, dst, table2[src])
    vs = dinv[:, None] * (G2 + T2F)
    z = f(vs) @ f(W2) + np.asarray(b2, np.float32)
    m = z.max(1, keepdims=True)
    return z - m - np.log(np.exp(z - m).sum(1, keepdims=True))


def build_nc(meta):
    import concourse.bacc as bacc
    import concourse.tile as tile
    import concourse.mybir as mybir

    dt = mybir.dt.float32
    bf = mybir.dt.bfloat16
    Alu = mybir.AluOpType
    Act = mybir.ActivationFunctionType
    B, nblk, nch = meta["B"], meta["nblk"], meta["nch"]

    nc = bacc.Bacc(None, target_bir_lowering=False, num_swdge_queues=NQ,
                   dynamic_dma_scratch_size=32768)
    p_xT = nc.declare_dram_parameter("xT", [F0, SHP], bf, isOutput=False)
    p_idx = [nc.declare_dram_parameter(f"idx{P}", [128, nch[P] * (CHUNK // 16)],
                                       mybir.dt.int16, isOutput=False)
             for P in range(2)]
    p_dl = [nc.declare_dram_parameter(f"dl{P}", [128, nblk[P]], dt,
                                      isOutput=False) for P in range(2)]
    p_deg = nc.declare_dram_parameter("deg", [128, NT], dt, isOutput=False)
    p_W1 = nc.declare_dram_parameter("W1", [F0, F1], bf, isOutput=False)
    p_W2 = nc.declare_dram_parameter("W2", [F1, F2], bf, isOutput=False)
    p_b1 = nc.declare_dram_parameter("b1b", [128, F1], dt, isOutput=False)
    p_b2 = nc.declare_dram_parameter("b2b", [128, F2], dt, isOutput=False)
    p_iota = nc.declare_dram_parameter("iota", [128, 256], bf, isOutput=False)
    p_ident = nc.declare_dram_parameter("ident", [128, 128], dt, isOutput=False)
    p_out = nc.declare_dram_parameter("out", [128, NT * F2], dt, isOutput=True)

    cc_in = [[nc.dram_tensor(f"cc_in{li}{P}", [128, (WA, WB)[P]], bf)
              for P in range(2)] for li in range(2)]
    cc_out = [[nc.dram_tensor(f"cc_out{li}{P}", [(EA, EB)[P], 128], bf,
                              addr_space="Shared")
               for P in range(2)] for li in range(2)]

    with tile.TileContext(nc) as tc:
        with (
            tc.tile_pool(name="cpool", bufs=1) as cpool,
            tc.tile_pool(name="spool", bufs=16) as spool,
            tc.tile_pool(name="stpool", bufs=24) as stpool,
            tc.tile_pool(name="wpool", bufs=4) as wpool,
            tc.tile_pool(name="ppool", bufs=4, space="PSUM") as ppool,
            tc.tile_pool(name="p2pool", bufs=2, space="PSUM") as p2pool,
        ):
            # ---- constants into SBUF
            xT = cpool.tile([F0, SHP], bf)
            nc.sync.dma_start(xT[:], p_xT[:])
            W1 = cpool.tile([F0, F1], bf)
            nc.sync.dma_start(W1[:], p_W1[:])
            W2 = cpool.tile([F1, F2], bf)
            nc.sync.dma_start(W2[:], p_W2[:])
            b1b = cpool.tile([128, F1], dt)
            nc.sync.dma_start(b1b[:], p_b1[:])
            b2b = cpool.tile([128, F2], dt)
            nc.sync.dma_start(b2b[:], p_b2[:])
            iota = cpool.tile([128, 256], bf)
            nc.sync.dma_start(iota[:], p_iota[:])
            ident = cpool.tile([128, 128], dt)
            nc.sync.dma_start(ident[:], p_ident[:])
            degt = cpool.tile([128, NT], dt)
            nc.sync.dma_start(degt[:], p_deg[:])
            idx_sb = []
            dl_sb = []
            for P in range(2):
                isb = cpool.tile([128, nch[P] * (CHUNK // 16)], mybir.dt.int16,
                                 name=f"isb{P}")
                nc.sync.dma_start(isb[:], p_idx[P][:])
                idx_sb.append(isb)
                dsb = cpool.tile([128, nblk[P]], dt, name=f"dsb{P}")
                nc.sync.dma_start(dsb[:], p_dl[P][:])
                dl_sb.append(dsb)

            recd = cpool.tile([128, NT], dt)
            nc.vector.reciprocal(recd[:], degt[:])
            dinv = cpool.tile([128, NT], dt)
            nc.scalar.activation(dinv[:], recd[:], Act.Sqrt)

            TshF = cpool.tile([128, NT * F1], dt)
            Tpad = cpool.tile([128, NT * F1], bf)
            T2F = cpool.tile([128, NT * F1], dt)
            T2pad = cpool.tile([128, NT * F1], bf)
            accA = cpool.tile([128, NT * F1], dt)
            accB = cpool.tile([128, NT * F1], dt)
            outsh = cpool.tile([128, NT * F2], dt)

            def fire_ag(li, P, pad_src):
                w0 = 0 if P == 0 else WA
                w1 = WA if P == 0 else WA + WB
                nc.sync.dma_start(cc_in[li][P][:], pad_src[:, w0:w1])
                nc.gpsimd.collective_compute(
                    "AllGather", Alu.bypass,
                    ins=[cc_in[li][P].ap().opt()],
                    outs=[cc_out[li][P].ap().opt()],
                    replica_groups=[list(range(CORES))])

            # ---- head: TshF = dinv * (x @ W1); Tpad gets bf16 copy
            for t in range(NT):
                psh = ppool.tile([128, F1], dt, tag="agg", name=f"psh{t}")
                nc.tensor.matmul(psh[:], xT[:, BLK * t:BLK * (t + 1)], W1[:],
                                 start=True, stop=True)
                nc.vector.tensor_scalar(
                    TshF[:, F1 * t:F1 * (t + 1)], psh[:], dinv[:, t:t + 1],
                    None, Alu.mult)
                nc.scalar.copy(Tpad[:, F1 * t:F1 * (t + 1)],
                               TshF[:, F1 * t:F1 * (t + 1)])
                if t == TA - 1:
                    fire_ag(0, 0, Tpad)
            fire_ag(0, 1, Tpad)

            qcounter = [0]

            def do_pass(li, P, tail_fn):
                table = cc_out[li][P]
                emitted = [0]
                chunks = {}

                def ensure_chunk(c):
                    while emitted[0] <= min(c + LOOKAHEAD, nch[P] - 1):
                        ce = emitted[0]
                        st = stpool.tile([128, CB, 128], bf, tag="st",
                                         name=f"st_l{li}p{P}c{ce}")
                        cols = CHUNK // 16
                        nc.gpsimd.dma_gather(
                            st[:], table[:],
                            idx_sb[P][:, ce * cols:(ce + 1) * cols],
                            CHUNK, CHUNK, 128,
                            single_packet=True,
                            queue_num=qcounter[0] % NQ)
                        qcounter[0] += 1
                        chunks[ce] = st
                        if ce >= LOOKAHEAD + 2:
                            chunks.pop(ce - LOOKAHEAD - 2, None)
                        emitted[0] += 1
                    return chunks[c]

                gb = 0
                for t in range(NT):
                    nb = int(B[P, t])
                    pagg = ppool.tile([128, F1], dt, tag="agg",
                                      name=f"pg{li}{P}_{t}")
                    for b in range(nb):
                        c, slot = gb // CB, gb % CB
                        st = ensure_chunk(c)
                        S = spool.tile([128, 256], bf, tag="S",
                                       name=f"S{li}{P}_{gb}")
                        nc.vector.tensor_scalar(
                            S[:], iota[:], dl_sb[P][:, gb:gb + 1], None,
                            Alu.is_equal)
                        nc.tensor.matmul(pagg[:], S[:, 0:128],
                                         st[:, slot, 0:F1],
                                         start=(b == 0), stop=False)
                        nc.tensor.matmul(pagg[:], S[:, 128:256],
                                         st[:, slot, F1:BLK],
                                         start=False, stop=(b == nb - 1))
                        gb += 1
                    tail_fn(t, pagg)

            # ---- layer 1 pass A: spill
            def spillA(t, pagg):
                nc.scalar.copy(accA[:, F1 * t:F1 * (t + 1)], pagg[:])

            do_pass(0, 0, spillA)

            # ---- layer 1 pass B: spill to accB, tails in a post-loop
            def spillB(t, pagg):
                nc.scalar.copy(accB[:, F1 * t:F1 * (t + 1)], pagg[:])

            do_pass(0, 1, spillB)

            uall = cpool.tile([128, NT * F1], dt)
            nc.vector.tensor_tensor(out=uall[:], in0=accB[:], in1=accA[:],
                                    op=Alu.add)
            nc.vector.tensor_tensor(out=uall[:], in0=uall[:], in1=TshF[:],
                                    op=Alu.add)
            for t in range(NT):
                e1 = wpool.tile([128, F1], dt, tag="e1", name=f"e1_{t}")
                nc.vector.scalar_tensor_tensor(
                    e1[:], uall[:, F1 * t:F1 * (t + 1)], dinv[:, t:t + 1],
                    b1b[:], Alu.mult, Alu.add)
                nc.vector.tensor_scalar(
                    T2F[:, F1 * t:F1 * (t + 1)], e1[:], 0.0, dinv[:, t:t + 1],
                    Alu.max, Alu.mult)
            for t in range(NT):
                nc.scalar.copy(T2pad[:, F1 * t:F1 * (t + 1)],
                               T2F[:, F1 * t:F1 * (t + 1)])
                if t == TA - 1:
                    fire_ag(1, 0, T2pad)
                if t == NT - 1:
                    fire_ag(1, 1, T2pad)

            # ---- layer 2 pass A
            do_pass(1, 0, spillA)

            # ---- layer 2 pass B: spill, then transpose, W2, log_softmax
            do_pass(1, 1, spillB)

            # stage 1 (DVE): vscAll = dinv * (accA + accB + T2F), into TshF buf
            vscAll = TshF
            nc.vector.tensor_tensor(out=vscAll[:], in0=accB[:], in1=accA[:],
                                    op=Alu.add)
            nc.vector.tensor_tensor(out=vscAll[:], in0=vscAll[:], in1=T2F[:],
                                    op=Alu.add)
            for t in range(NT):
                nc.vector.tensor_scalar(
                    vscAll[:, F1 * t:F1 * (t + 1)],
                    vscAll[:, F1 * t:F1 * (t + 1)],
                    dinv[:, t:t + 1], None, Alu.mult)
            # stage 2/3 (PE transpose + ACT copy) -> zTall
            zTall = cpool.tile([F1, NT * 128], bf)
            for t in range(NT):
                trp = p2pool.tile([F1, 128], dt, tag="tr", name=f"tr_{t}")
                nc.tensor.transpose(trp[:], vscAll[:, F1 * t:F1 * (t + 1)],
                                    ident[:])
                nc.scalar.copy(zTall[:, 128 * t:128 * (t + 1)], trp[:])
            # stage 4/5 (PE W2 matmul + DVE bias) -> e4all
            e4all = cpool.tile([128, NT * F2], dt)
            for t in range(NT):
                po = p2pool.tile([128, F2], dt, tag="po", name=f"po_{t}")
                nc.tensor.matmul(po[:], zTall[:, 128 * t:128 * (t + 1)],
                                 W2[:], start=True, stop=True)
                nc.vector.tensor_tensor(
                    out=e4all[:, F2 * t:F2 * (t + 1)], in0=po[:], in1=b2b[:],
                    op=Alu.add)
            # stage 6: log_softmax, stage-major
            mAll = cpool.tile([128, NT], dt)
            for t in range(NT):
                nc.vector.tensor_reduce(
                    mAll[:, t:t + 1], e4all[:, F2 * t:F2 * (t + 1)],
                    axis=mybir.AxisListType.X, op=Alu.max)
            nmAll = cpool.tile([128, NT], dt)
            nc.vector.tensor_scalar(nmAll[:], mAll[:], -1.0, None, Alu.mult)
            smAll = cpool.tile([128, NT], dt)
            for t in range(NT):
                ex = wpool.tile([128, F2], dt, tag="ex", name=f"ex_{t}")
                nc.scalar.activation(ex[:], e4all[:, F2 * t:F2 * (t + 1)],
                                     Act.Exp, bias=nmAll[:, t:t + 1])
                nc.vector.tensor_reduce(
                    smAll[:, t:t + 1], ex[:], axis=mybir.AxisListType.X,
                    op=Alu.add)
            lgAll = cpool.tile([128, NT], dt)
            nc.scalar.activation(lgAll[:], smAll[:], Act.Ln)
            for t in range(NT):
                nc.vector.tensor_scalar(
                    outsh[:, F2 * t:F2 * (t + 1)],
                    e4all[:, F2 * t:F2 * (t + 1)], mAll[:, t:t + 1],
                    lgAll[:, t:t + 1], Alu.subtract, Alu.subtract)
            nc.sync.dma_start(p_out[:], outsh[:])

    nc.finalize()
    return nc


LAST_EXEC_NS = None


def kernel(x, edge_index, W1, b1, W2, b2):
    from concourse.bass_utils import run_bass_kernel_spmd

    x = np.asarray(x, np.float32)
    data, consts, meta = host_prep(x, np.asarray(edge_index), W1, b1, W2, b2)
    nc = build_nc(meta)
    in_maps = []
    for i in range(CORES):
        m = dict(data[i])
        m.update({k: np.ascontiguousarray(v) for k, v in consts.items()})
        in_maps.append(m)
    import os as _os
    trace = bool(int(_os.environ.get("GCN_TRACE", "0")))
    res = run_bass_kernel_spmd(nc, in_maps, core_ids=list(range(CORES)),
                               trace=trace)
    global LAST_EXEC_NS
    LAST_EXEC_NS = res.exec_time_ns
    if trace and res.instructions_and_trace:
        try:
            import pickle
            insts, tpath = res.instructions_and_trace
            with open("/tmp/gcn_insts.pkl", "wb") as f:
                pickle.dump({"insts": insts, "exec_ns": res.exec_time_ns,
                             "trace_path": tpath}, f)
        except Exception as e:
            print("trace stash failed:", e)
    outs = []
    for i in range(CORES):
        o = res.results[i]["out"]  # [128, NT*F2]
        outs.append(o.reshape(128, NT, F2).transpose(1, 0, 2).reshape(SHP, F2))
    res_full = np.zeros((N_NODES, F2), np.float32)
    for i in range(CORES):
        res_full[SH * i:SH * (i + 1)] = outs[i][:SH]
    return res_full


def replay_check(inputs, data, meta, core=3):
    """Replay core `core`'s layer-1 streams against a direct edge sum."""
    x, ei = inputs["x"], inputs["edge_index"]
    W1 = inputs["W1"]
    src, dst = np.asarray(ei[0]), np.asarray(ei[1])
    deg = np.bincount(dst, minlength=N_NODES).astype(np.float32) + 1.0
    dinv = 1.0 / np.sqrt(deg)
    h1 = _bf(x).astype(np.float32) @ _bf(W1).astype(np.float32)
    TshF = dinv[:, None] * h1
    tb = _bf(TshF).astype(np.float32)
    # padded-row table per shard [128, WA] / [128, WB]
    ccA = np.zeros((CORES, BLK, WA), np.float32)
    ccB = np.zeros((CORES, BLK, WB), np.float32)
    for s in range(CORES):
        sh = np.zeros((SHP, F1), np.float32)
        sh[:SH] = tb[SH * s:SH * (s + 1)]
        g = sh.reshape(NT, BLK, F1)
        ccA[s] = g[:TA].transpose(1, 0, 2).reshape(BLK, WA)
        ccB[s] = g[TA:].transpose(1, 0, 2).reshape(BLK, WB)
    elemsA = ccA.reshape(-1)  # flat bf16-unit stream
    elemsB = ccB.reshape(-1)
    EAr = elemsA.reshape(EA, BLK)
    EBr = elemsB.reshape(EB, BLK)

    d = data[core]
    B = meta["B"]
    agg = np.zeros((NT, BLK, F1), np.float32)
    for P in range(2):
        tab = EAr if P == 0 else EBr
        idxp = d[f"idx{P}"]
        stream = idxp[:16].T.reshape(-1).astype(np.int64)
        dlp = d[f"dl{P}"]
        gb = 0
        for t in range(NT):
            for b in range(int(B[P, t])):
                rows = stream[gb * BLK:(gb + 1) * BLK]
                G = tab[rows]                      # [128, 128]
                dl = dlp[:, gb]
                for e in range(BLK):
                    dv = int(dl[e])
                    if dv >= 2 * BLK:
                        continue
                    par, dd = dv // BLK, dv % BLK
                    agg[t, dd] += G[e, F1 * par:F1 * par + F1]
                gb += 1
    # ground truth for this core's shard
    G1 = np.zeros((N_NODES, F1), np.float32)
    np.add.at(G1, dst, tb[src])
    gt = np.zeros((SHP, F1), np.float32)
    gt[:SH] = G1[SH * core:SH * (core + 1)]
    got = agg.reshape(SHP, F1)
    err = np.abs(got - gt).max()
    print(f"replay: max abs err {err:.3e} (scale {np.abs(gt).max():.2f})")
    assert err < 2e-2, "stream replay mismatch"


if __name__ == "__main__":
    z = np.load("/tmp/gcn_ref.npz")
    inputs = {k: z[k] for k in z.files if k != "expected"}
    expected = z["expected"]
    data, consts, meta = host_prep(**inputs)
    print("nblk:", meta["nblk"], "nch:", meta["nch"])
    got = numpy_sim(**inputs)
    err = np.abs(got - expected)
    rel = err.max() / np.abs(expected).max()
    print(f"numpy-sim (bf16 emul) max abs err {err.max():.3e}  rel {rel:.3e}")
    replay_check(inputs, data, meta)


# revision 15
# speedup vs baseline: 1.1453x; 1.0694x over previous
"""GCN (2-layer) Trainium2 kernel over 8 NeuronCores — v2.

Strategy (dst-sharded pull-gather, bf16):
- Nodes sharded contiguously: core i owns nodes [6250*i, 6250*(i+1)).
- Layer table rows are bf16, PADDED to 128 values (64 real + 64 unread) so
  every row is one 256-byte gather element (dma_gather requires 256B-aligned
  elements; gather cost is per-descriptor latency, so padding is free).
- The table is AllGather'd in two halves (src tiles 0:25 / 25:49) so the
  second half's collective overlaps the first half's gather+aggregate pass.
- Aggregation per 128-dst tile: gathered 128-edge blocks are summed with a
  TensorE matmul against a DVE-built one-hot selector S (bf16).
- Self-loop terms are folded into the per-tile tail math (never gathered).
- Gathers run in 2048-index chunks round-robined over 4 SWDGE queues.
- Layer 2 aggregates the (dinv*relu(.)) table, then applies W2 after a PE
  transpose; log_softmax on ACT+DVE.
The edge structure is baked into the program; block counts are padded to the
max over cores so the SPMD program is identical on all 8 cores.
"""

import numpy as np

try:
    from ml_dtypes import bfloat16 as bf16np
except ImportError:  # pragma: no cover
    bf16np = None

N_NODES = 50000
CORES = 8
SH = 6250          # owned nodes per core
SHP = 6272         # padded shard rows (49*128)
NT = 49            # dst tiles per core
BLK = 128
F0, F1, F2 = 96, 64, 16
TA = 25            # tiles in half A
TB = NT - TA       # 24
WA, WB = TA * F1, TB * F1            # compact cols per half: 1600 / 1536
EA, EB = 1024 * WA // BLK, 1024 * WB // BLK  # 256B elements per half: 12800 / 12288
CHUNK = 1024
CB = CHUNK // BLK  # blocks per chunk
NQ = 4             # SWDGE queues
PAD_DL = 300.0     # is_equal miss => zero S row
LOOKAHEAD = 20


def _bf(x):
    return np.asarray(x, np.float32).astype(bf16np)


def host_prep(x, edge_index, W1, b1, W2, b2):
    src = np.asarray(edge_index[0], dtype=np.int64)
    dst = np.asarray(edge_index[1], dtype=np.int64)
    deg_full = np.bincount(dst, minlength=N_NODES).astype(np.float32) + 1.0

    # split edges by dst shard
    order = np.argsort(dst, kind="stable")
    s_sorted, d_sorted = src[order], dst[order]
    bounds = np.searchsorted(d_sorted, np.arange(0, N_NODES + 1, SH))

    # per-core, per-pass, per-tile edge lists: (elem, dl)
    counts = np.zeros((CORES, 2, NT), np.int64)
    lists = [[[None] * NT for _ in range(2)] for _ in range(CORES)]
    for i in range(CORES):
        es = s_sorted[bounds[i]:bounds[i + 1]]
        ed = d_sorted[bounds[i]:bounds[i + 1]] - SH * i
        s_sh = es // SH
        l = es - SH * s_sh
        ts = l // BLK
        p = l - ts * BLK
        half = (ts >= TA).astype(np.int64)
        rowA = WA * (s_sh * BLK + p) + F1 * ts          # bf16 units
        rowB = WB * (s_sh * BLK + p) + F1 * (ts - TA)
        elem = np.where(half == 0, rowA // BLK, rowB // BLK)
        par = np.where(half == 0, (rowA // F1) % 2, (rowB // F1) % 2)
        t = ed // BLK
        dl = ed - t * BLK + BLK * par                    # 0..255
        key = half * NT + t
        o = np.argsort(key, kind="stable")
        key_s, elem_s, dl_s = key[o], elem[o], dl[o]
        kb = np.searchsorted(key_s, np.arange(2 * NT + 1))
        for P in range(2):
            for tt in range(NT):
                a, b = kb[P * NT + tt], kb[P * NT + tt + 1]
                lists[i][P][tt] = (elem_s[a:b], dl_s[a:b])
                counts[i, P, tt] = b - a

    # uniform block counts across cores
    B = np.maximum(1, -(-counts.max(axis=0) // BLK))  # [2, NT]
    nblk = [int(B[P].sum()) for P in range(2)]
    nch = [-(-nblk[P] // CB) for P in range(2)]

    data = []
    for i in range(CORES):
        d = dict()
        for P in range(2):
            els, dls = [], []
            for tt in range(NT):
                e, q = lists[i][P][tt]
                pad = int(B[P, tt]) * BLK - len(e)
                els.append(np.concatenate([e, np.zeros(pad, np.int64)]))
                dls.append(np.concatenate([q.astype(np.float32),
                                           np.full(pad, PAD_DL, np.float32)]))
            estream = np.concatenate(els)
            dstream = np.concatenate(dls)
            tail = nch[P] * CHUNK - len(estream)
            estream = np.concatenate([estream, np.zeros(tail, np.int64)])
            # idx plane [128, nch*128]: idx j at [j%16, j//16], replicated x8
            pl = estream.reshape(-1, 16).T.astype(np.int16)
            d[f"idx{P}"] = np.ascontiguousarray(np.tile(pl, (8, 1)))
            # dl plane [128, nblk]
            d[f"dl{P}"] = np.ascontiguousarray(
                dstream.reshape(-1, BLK).T.astype(np.float32))
        degp = np.ones((BLK, NT), np.float32)
        dsh = deg_full[SH * i:SH * (i + 1)]
        dp = np.concatenate([dsh, np.ones(SHP - SH, np.float32)])
        degp[:, :] = dp.reshape(NT, BLK).T
        d["deg"] = np.ascontiguousarray(degp)
        xs = np.zeros((F0, SHP), np.float32)
        xs[:, :SH] = np.asarray(x[SH * i:SH * (i + 1)], np.float32).T
        d["xT"] = np.ascontiguousarray(_bf(xs))
        data.append(d)

    ident = np.eye(BLK, dtype=np.float32)
    consts = dict(
        W1=_bf(W1), W2=_bf(W2),
        b1b=np.tile(np.asarray(b1, np.float32), (BLK, 1)),
        b2b=np.tile(np.asarray(b2, np.float32), (BLK, 1)),
        iota=_bf(np.tile(np.arange(2 * BLK, dtype=np.float32), (BLK, 1))),
        ident=ident,
    )
    meta = dict(B=B, nblk=nblk, nch=nch)
    return data, consts, meta


def numpy_sim(x, edge_index, W1, b1, W2, b2):
    """Emulate the device numerics (bf16 tables/weights) edge-wise."""
    def f(a):
        return _bf(a).astype(np.float32)

    src = np.asarray(edge_index[0]); dst = np.asarray(edge_index[1])
    deg = np.bincount(dst, minlength=N_NODES).astype(np.float32) + 1.0
    dinv = 1.0 / np.sqrt(deg)
    h1 = f(x) @ f(W1)  # f32 accum of bf16 operands
    TshF = dinv[:, None] * h1
    table1 = f(TshF)
    G1 = np.zeros_like(TshF)
    np.add.at(G1, dst, table1[src])
    e1 = dinv[:, None] * (G1 + TshF) + np.asarray(b1, np.float32)
    T2F = dinv[:, None] * np.maximum(e1, 0.0)
    table2 = f(T2F)
    G2 = np.zeros_like(T2F)
    np.add.at(G2, dst, table2[src])
    vs = dinv[:, None] * (G2 + T2F)
    z = f(vs) @ f(W2) + np.asarray(b2, np.float32)
    m = z.max(1, keepdims=True)
    return z - m - np.log(np.exp(z - m).sum(1, keepdims=True))


def build_nc(meta):
    import concourse.bacc as bacc
    import concourse.tile as tile
    import concourse.mybir as mybir

    dt = mybir.dt.float32
    bf = mybir.dt.bfloat16
    Alu = mybir.AluOpType
    Act = mybir.ActivationFunctionType
    B, nblk, nch = meta["B"], meta["nblk"], meta["nch"]

    nc = bacc.Bacc(None, target_bir_lowering=False, num_swdge_queues=NQ,
                   dynamic_dma_scratch_size=32768)
    p_xT = nc.declare_dram_parameter("xT", [F0, SHP], bf, isOutput=False)
    p_idx = [nc.declare_dram_parameter(f"idx{P}", [128, nch[P] * (CHUNK // 16)],
                                       mybir.dt.int16, isOutput=False)
             for P in range(2)]
    p_dl = [nc.declare_dram_parameter(f"dl{P}", [128, nblk[P]], dt,
                                      isOutput=False) for P in range(2)]
    p_deg = nc.declare_dram_parameter("deg", [128, NT], dt, isOutput=False)
    p_W1 = nc.declare_dram_parameter("W1", [F0, F1], bf, isOutput=False)
    p_W2 = nc.declare_dram_parameter("W2", [F1, F2], bf, isOutput=False)
    p_b1 = nc.declare_dram_parameter("b1b", [128, F1], dt, isOutput=False)
    p_b2 = nc.declare_dram_parameter("b2b", [128, F2], dt, isOutput=False)
    p_iota = nc.declare_dram_parameter("iota", [128, 256], bf, isOutput=False)
    p_ident = nc.declare_dram_parameter("ident", [128, 128], dt, isOutput=False)
    p_out = nc.declare_dram_parameter("out", [128, NT * F2], dt, isOutput=True)

    cc_in = [[nc.dram_tensor(f"cc_in{li}{P}", [128, (WA, WB)[P]], bf)
              for P in range(2)] for li in range(2)]
    cc_out = [[nc.dram_tensor(f"cc_out{li}{P}", [(EA, EB)[P], 128], bf,
                              addr_space="Shared")
               for P in range(2)] for li in range(2)]

    with tile.TileContext(nc) as tc:
        with (
            tc.tile_pool(name="cpool", bufs=1) as cpool,
            tc.tile_pool(name="spool", bufs=16) as spool,
            tc.tile_pool(name="stpool", bufs=24) as stpool,
            tc.tile_pool(name="wpool", bufs=4) as wpool,
            tc.tile_pool(name="ppool", bufs=4, space="PSUM") as ppool,
            tc.tile_pool(name="p2pool", bufs=2, space="PSUM") as p2pool,
        ):
            # ---- constants into SBUF
            xT = cpool.tile([F0, SHP], bf)
            nc.sync.dma_start(xT[:], p_xT[:])
            W1 = cpool.tile([F0, F1], bf)
            nc.sync.dma_start(W1[:], p_W1[:])
            W2 = cpool.tile([F1, F2], bf)
            nc.sync.dma_start(W2[:], p_W2[:])
            b1b = cpool.tile([128, F1], dt)
            nc.sync.dma_start(b1b[:], p_b1[:])
            b2b = cpool.tile([128, F2], dt)
            nc.sync.dma_start(b2b[:], p_b2[:])
            iota = cpool.tile([128, 256], bf)
            nc.sync.dma_start(iota[:], p_iota[:])
            ident = cpool.tile([128, 128], dt)
            nc.sync.dma_start(ident[:], p_ident[:])
            degt = cpool.tile([128, NT], dt)
            nc.sync.dma_start(degt[:], p_deg[:])
            idx_sb = []
            dl_sb = []
            for P in range(2):
                isb = cpool.tile([128, nch[P] * (CHUNK // 16)], mybir.dt.int16,
                                 name=f"isb{P}")
                idx_sb.append(isb)
                dsb = cpool.tile([128, nblk[P]], dt, name=f"dsb{P}")
                dl_sb.append(dsb)

            recd = cpool.tile([128, NT], dt)
            nc.vector.reciprocal(recd[:], degt[:])
            dinv = cpool.tile([128, NT], dt)
            nc.scalar.activation(dinv[:], recd[:], Act.Sqrt)

            TshF = cpool.tile([128, NT * F1], dt)
            Tpad = cpool.tile([128, NT * F1], bf)
            T2F = cpool.tile([128, NT * F1], dt)
            T2pad = cpool.tile([128, NT * F1], bf)
            accA = cpool.tile([128, NT * F1], dt)
            accB = cpool.tile([128, NT * F1], dt)
            outsh = cpool.tile([128, NT * F2], dt)

            def fire_ag(li, P, pad_src):
                w0 = 0 if P == 0 else WA
                w1 = WA if P == 0 else WA + WB
                nc.sync.dma_start(cc_in[li][P][:], pad_src[:, w0:w1])
                nc.gpsimd.collective_compute(
                    "AllGather", Alu.bypass,
                    ins=[cc_in[li][P].ap().opt()],
                    outs=[cc_out[li][P].ap().opt()],
                    replica_groups=[list(range(CORES))])

            # ---- head: TshF = dinv * (x @ W1); Tpad gets bf16 copy
            for t in range(NT):
                psh = ppool.tile([128, F1], dt, tag="agg", name=f"psh{t}")
                nc.tensor.matmul(psh[:], xT[:, BLK * t:BLK * (t + 1)], W1[:],
                                 start=True, stop=True)
                nc.vector.tensor_scalar(
                    TshF[:, F1 * t:F1 * (t + 1)], psh[:], dinv[:, t:t + 1],
                    None, Alu.mult)
                nc.scalar.copy(Tpad[:, F1 * t:F1 * (t + 1)],
                               TshF[:, F1 * t:F1 * (t + 1)])
                if t == TA - 1:
                    fire_ag(0, 0, Tpad)
            fire_ag(0, 1, Tpad)
            for P in range(2):
                nc.sync.dma_start(idx_sb[P][:], p_idx[P][:])
                nc.sync.dma_start(dl_sb[P][:], p_dl[P][:])

            qcounter = [0]

            def do_pass(li, P, tail_fn):
                table = cc_out[li][P]
                emitted = [0]
                chunks = {}

                def ensure_chunk(c):
                    while emitted[0] <= min(c + LOOKAHEAD, nch[P] - 1):
                        ce = emitted[0]
                        st = stpool.tile([128, CB, 128], bf, tag="st",
                                         name=f"st_l{li}p{P}c{ce}")
                        cols = CHUNK // 16
                        nc.gpsimd.dma_gather(
                            st[:], table[:],
                            idx_sb[P][:, ce * cols:(ce + 1) * cols],
                            CHUNK, CHUNK, 128,
                            single_packet=True,
                            queue_num=qcounter[0] % NQ)
                        qcounter[0] += 1
                        chunks[ce] = st
                        if ce >= LOOKAHEAD + 2:
                            chunks.pop(ce - LOOKAHEAD - 2, None)
                        emitted[0] += 1
                    return chunks[c]

                gb = 0
                for t in range(NT):
                    nb = int(B[P, t])
                    pagg = ppool.tile([128, F1], dt, tag="agg",
                                      name=f"pg{li}{P}_{t}")
                    for b in range(nb):
                        c, slot = gb // CB, gb % CB
                        st = ensure_chunk(c)
                        S = spool.tile([128, 256], bf, tag="S",
                                       name=f"S{li}{P}_{gb}")
                        nc.vector.tensor_scalar(
                            S[:], iota[:], dl_sb[P][:, gb:gb + 1], None,
                            Alu.is_equal)
                        nc.tensor.matmul(pagg[:], S[:, 0:128],
                                         st[:, slot, 0:F1],
                                         start=(b == 0), stop=False)
                        nc.tensor.matmul(pagg[:], S[:, 128:256],
                                         st[:, slot, F1:BLK],
                                         start=False, stop=(b == nb - 1))
                        gb += 1
                    tail_fn(t, pagg)

            # ---- layer 1 pass A: spill
            def spillA(t, pagg):
                nc.scalar.copy(accA[:, F1 * t:F1 * (t + 1)], pagg[:])

            do_pass(0, 0, spillA)

            # ---- layer 1 pass B: spill to accB, tails in a post-loop
            def spillB(t, pagg):
                nc.scalar.copy(accB[:, F1 * t:F1 * (t + 1)], pagg[:])

            do_pass(0, 1, spillB)

            uall = cpool.tile([128, NT * F1], dt)
            nc.vector.tensor_tensor(out=uall[:], in0=accB[:], in1=accA[:],
                                    op=Alu.add)
            nc.vector.tensor_tensor(out=uall[:], in0=uall[:], in1=TshF[:],
                                    op=Alu.add)
            for t in range(NT):
                e1 = wpool.tile([128, F1], dt, tag="e1", name=f"e1_{t}")
                nc.vector.scalar_tensor_tensor(
                    e1[:], uall[:, F1 * t:F1 * (t + 1)], dinv[:, t:t + 1],
                    b1b[:], Alu.mult, Alu.add)
                nc.vector.tensor_scalar(
                    T2F[:, F1 * t:F1 * (t + 1)], e1[:], 0.0, dinv[:, t:t + 1],
                    Alu.max, Alu.mult)
            for t in range(NT):
                nc.scalar.copy(T2pad[:, F1 * t:F1 * (t + 1)],
                               T2F[:, F1 * t:F1 * (t + 1)])
                if t == TA - 1:
                    fire_ag(1, 0, T2pad)
                if t == NT - 1:
                    fire_ag(1, 1, T2pad)

            # ---- layer 2 pass A
            do_pass(1, 0, spillA)

            # ---- layer 2 pass B: spill, then transpose, W2, log_softmax
            do_pass(1, 1, spillB)

            # stage 1 (DVE): vscAll = dinv * (accA + accB + T2F), into TshF buf
            vscAll = TshF
            nc.vector.tensor_tensor(out=vscAll[:], in0=accB[:], in1=accA[:],
                                    op=Alu.add)
            nc.vector.tensor_tensor(out=vscAll[:], in0=vscAll[:], in1=T2F[:],
                                    op=Alu.add)
            for t in range(NT):
                nc.vector.tensor_scalar(
                    vscAll[:, F1 * t:F1 * (t + 1)],
                    vscAll[:, F1 * t:F1 * (t + 1)],
                    dinv[:, t:t + 1], None, Alu.mult)
            # stage 2/3 (PE transpose + ACT copy) -> zTall
            zTall = cpool.tile([F1, NT * 128], bf)
            for t in range(NT):
                trp = p2pool.tile([F1, 128], dt, tag="tr", name=f"tr_{t}")
                nc.tensor.transpose(trp[:], vscAll[:, F1 * t:F1 * (t + 1)],
                                    ident[:])
                nc.scalar.copy(zTall[:, 128 * t:128 * (t + 1)], trp[:])
            # stage 4/5 (PE W2 matmul + DVE bias) -> e4all
            e4all = cpool.tile([128, NT * F2], dt)
            for t in range(NT):
                po = p2pool.tile([128, F2], dt, tag="po", name=f"po_{t}")
                nc.tensor.matmul(po[:], zTall[:, 128 * t:128 * (t + 1)],
                                 W2[:], start=True, stop=True)
                nc.vector.tensor_tensor(
                    out=e4all[:, F2 * t:F2 * (t + 1)], in0=po[:], in1=b2b[:],
                    op=Alu.add)
            # stage 6: log_softmax, stage-major
            mAll = cpool.tile([128, NT], dt)
            for t in range(NT):
                nc.vector.tensor_reduce(
                    mAll[:, t:t + 1], e4all[:, F2 * t:F2 * (t + 1)],
                    axis=mybir.AxisListType.X, op=Alu.max)
            nmAll = cpool.tile([128, NT], dt)
            nc.vector.tensor_scalar(nmAll[:], mAll[:], -1.0, None, Alu.mult)
            smAll = cpool.tile([128, NT], dt)
            for t in range(NT):
                ex = wpool.tile([128, F2], dt, tag="ex", name=f"ex_{t}")
                nc.scalar.activation(ex[:], e4all[:, F2 * t:F2 * (t + 1)],
                                     Act.Exp, bias=nmAll[:, t:t + 1])
                nc.vector.tensor_reduce(
                    smAll[:, t:t + 1], ex[:], axis=mybir.AxisListType.X,
                    op=Alu.add)
            lgAll = cpool.tile([128, NT], dt)
            nc.scalar.activation(lgAll[:], smAll[:], Act.Ln)
            for t in range(NT):
                nc.vector.tensor_scalar(
                    outsh[:, F2 * t:F2 * (t + 1)],
                    e4all[:, F2 * t:F2 * (t + 1)], mAll[:, t:t + 1],
                    lgAll[:, t:t + 1], Alu.subtract, Alu.subtract)
            nc.sync.dma_start(p_out[:], outsh[:])

    nc.finalize()
    return nc


LAST_EXEC_NS = None


def kernel(x, edge_index, W1, b1, W2, b2):
    from concourse.bass_utils import run_bass_kernel_spmd

    x = np.asarray(x, np.float32)
    data, consts, meta = host_prep(x, np.asarray(edge_index), W1, b1, W2, b2)
    nc = build_nc(meta)
    in_maps = []
    for i in range(CORES):
        m = dict(data[i])
        m.update({k: np.ascontiguousarray(v) for k, v in consts.items()})
        in_maps.append(m)
    import os as _os
    trace = bool(int(_os.environ.get("GCN_TRACE", "0")))
    res = run_bass_kernel_spmd(nc, in_maps, core_ids=list(range(CORES)),
                               trace=trace)
    global LAST_EXEC_NS
    LAST_EXEC_NS = res.exec_time_ns
    if trace and res.instructions_and_trace:
        try:
            import pickle
            insts, tpath = res.instructions_and_trace
            with open("/tmp/gcn_insts.pkl", "wb") as f:
                pickle.dump({"insts": insts, "exec_ns": res.exec_time_ns,
                             "trace_path": tpath}, f)
        except Exception as e:
            print("trace stash failed:", e)
    outs = []
    for i in range(CORES):
        o = res.results[i]["out"]  # [128, NT*F2]
        outs.append(o.reshape(128, NT, F2).transpose(1, 0, 2).reshape(SHP, F2))
    res_full = np.zeros((N_NODES, F2), np.float32)
    for i in range(CORES):
        res_full[SH * i:SH * (i + 1)] = outs[i][:SH]
    return res_full


def replay_check(inputs, data, meta, core=3):
    """Replay core `core`'s layer-1 streams against a direct edge sum."""
    x, ei = inputs["x"], inputs["edge_index"]
    W1 = inputs["W1"]
    src, dst = np.asarray(ei[0]), np.asarray(ei[1])
    deg = np.bincount(dst, minlength=N_NODES).astype(np.float32) + 1.0
    dinv = 1.0 / np.sqrt(deg)
    h1 = _bf(x).astype(np.float32) @ _bf(W1).astype(np.float32)
    TshF = dinv[:, None] * h1
    tb = _bf(TshF).astype(np.float32)
    # padded-row table per shard [128, WA] / [128, WB]
    ccA = np.zeros((CORES, BLK, WA), np.float32)
    ccB = np.zeros((CORES, BLK, WB), np.float32)
    for s in range(CORES):
        sh = np.zeros((SHP, F1), np.float32)
        sh[:SH] = tb[SH * s:SH * (s + 1)]
        g = sh.reshape(NT, BLK, F1)
        ccA[s] = g[:TA].transpose(1, 0, 2).reshape(BLK, WA)
        ccB[s] = g[TA:].transpose(1, 0, 2).reshape(BLK, WB)
    elemsA = ccA.reshape(-1)  # flat bf16-unit stream
    elemsB = ccB.reshape(-1)
    EAr = elemsA.reshape(EA, BLK)
    EBr = elemsB.reshape(EB, BLK)

    d = data[core]
    B = meta["B"]
    agg = np.zeros((NT, BLK, F1), np.float32)
    for P in range(2):
        tab = EAr if P == 0 else EBr
        idxp = d[f"idx{P}"]
        stream = idxp[:16].T.reshape(-1).astype(np.int64)
        dlp = d[f"dl{P}"]
        gb = 0
        for t in range(NT):
            for b in range(int(B[P, t])):
                rows = stream[gb * BLK:(gb + 1) * BLK]
                G = tab[rows]                      # [128, 128]
                dl = dlp[:, gb]
                for e in range(BLK):
                    dv = int(dl[e])
                    if dv >= 2 * BLK:
                        continue
                    par, dd = dv // BLK, dv % BLK
                    agg[t, dd] += G[e, F1 * par:F1 * par + F1]
                gb += 1
    # ground truth for this core's shard
    G1 = np.zeros((N_NODES, F1), np.float32)
    np.add.at(G1, dst, tb[src])
    gt = np.zeros((SHP, F1), np.float32)
    gt[:SH] = G1[SH * core:SH * (core + 1)]
    got = agg.reshape(SHP, F1)
    err = np.abs(got - gt).max()
    print(f"replay: max abs err {err:.3e} (scale {np.abs(gt).max():.2f})")
    assert err < 2e-2, "stream replay mismatch"


if __name__ == "__main__":
    z = np.load("/tmp/gcn_ref.npz")
    inputs = {k: z[k] for k in z.files if k != "expected"}
    expected = z["expected"]
    data, consts, meta = host_prep(**inputs)
    print("nblk:", meta["nblk"], "nch:", meta["nch"])
    got = numpy_sim(**inputs)
    err = np.abs(got - expected)
    rel = err.max() / np.abs(expected).max()
    print(f"numpy-sim (bf16 emul) max abs err {err.max():.3e}  rel {rel:.3e}")
    replay_check(inputs, data, meta)
